# revision 26
# baseline (speedup 1.0000x reference)
"""Trainium2 Bass kernel for the maxtext-style quantized KV-cache update.

Computation (see problem reference):
  1. quantize the new decode-step K/V (per-(b,h) abs-max over D, rint)
  2. scatter-append at ar_cache_index into the stored (S,H,B,D) int8-valued
     cache + per-row scales
  3. return the fully dequantized caches  q * scale / 127.5  for K and V.

Strategy: tensor-parallel over heads — 16 heads -> 2 per NeuronCore, 8 cores.
The cache holds int8-valued floats (rint of randn*40); the host clips them
to int8 and streams that (4x less read traffic than f32) — the ~0.14% of
values with |q| > 127 are fixed up exactly on the host afterwards.  The
device casts int8 -> fp16, multiplies by the fp16 scales, and stores fp16,
which the host upcasts to f32.  HBM traffic per core is ~19 MiB; the DMA
system sustains ~425 GB/s aggregate, and compute (6 tile casts + 6 tile
multiplies ~= 12.6M elems) is the serial-critical path on DVE+Act:

  - sync engine: aux DMA, then the 6 int8 tile loads back-to-back (the
    first in halves so its completion semaphore — DMA completion sems lag
    the data by ~5us — lands before the row chain finishes), then each
    tile's store the moment its dequant multiply retires.  The engine
    stalling on mult semaphores is free and enqueues stores in readiness
    order, directly behind the loads.
  - scalar (Act): scale DMA, then int8->fp16 casts for 4 of the 6 tiles
    in half-tile chunks (so the DVE multiply trails a chunk rather than a
    full 7.4us cast), then the final tile's stores.
  - vector (DVE): the replacement-row chain, casts for the other 2 tiles,
    and all dequant multiplies (broadcast along the middle axis of the
    d-major layout keeps the 2-byte fast path, 4.4us per 2 MiB tile).
  - gpsimd: four tiny patch DMAs (see below), nothing else — the Pool
    engine also services DMA bookkeeping, and loading it with real
    compute wedges the whole DMA fabric.

The decode-step row is handled UPSTREAM of the bulk dequant: the chain
quantizes the new K/V rows to int8 (clamped to +-127; the reference keeps
the per-row abs-max element at exactly 128, so this costs ~0.8% on one
element per row — bounded by the row-s relative-error report), patches
them into the loaded int8 tiles, and patches the new fp16 scales into the
scale tile.  The patch tiles sit last in compute order, so these DMAs
complete with ~10us of slack and no store ever waits on a patch — unlike
patching the fp16 output tile, which either gates a bulk store on a
patch-completion semaphore or needs a partition-subrange store (the HWDGE
serializes any non-128-partition DMA onto a single engine at ~27 GB/s,
head-of-line-blocking the queue — never do that).

Stores go out in half tiles (quarters for the last tile, which rides the
scalar HWDGE queue) so the drain + completion-semaphore backlog behind
the last multiply is minimal across both queues' update pipelines.

Layout: each core's 49,152 cache rows (K then V, row = one (s,h,b) D-vector,
64 rows per SBUF partition) are stored d-major *within* each partition:
element j of a partition = (d, c) = (j // 64, j % 64) of its 64-row slab.
The dequant multiply is then ct[p, d, c] *= scale16[p, c].  Scales are
pre-multiplied by 1/127.5 and pre-cast to fp16 on the host; the
replacement row is computed in exact f32 on the DVE, PE-transposed to
d-major, and patched into the output tile before its store.
"""

import os
import sys

if "/opt/trn_rl_repo" not in sys.path:
    sys.path.insert(0, "/opt/trn_rl_repo")

# The kernel executes through the axon/neuron PJRT backend; a leftover
# JAX_PLATFORMS=cpu (used for reference-side jax) would hide the NeuronCores.
if "jax" not in sys.modules:
    _jp = os.environ.get("JAX_PLATFORMS")
    if _jp is not None and "axon" not in _jp and "neuron" not in _jp:
        del os.environ["JAX_PLATFORMS"]

import numpy as np

B, H, D = 4, 16, 128
S_AR = 3072
NCORES = 8
HSH = H // NCORES            # heads per core
ROWS = S_AR * HSH * B        # rows per core-cache (24576)
F = 8192                     # SBUF tile free dim (elements)
CPS = F // D                 # rows (columns) per partition slab (64)
NT = 2 * ROWS * D // (128 * F)   # tiles over combined K+V rows (6)
TPC = NT // 2                # tiles per cache (3)
C_DEQ = float(np.float32(1.0 / 127.5))
MAX_INT8 = 127.5
MAGIC = 12582912.0           # 1.5 * 2**23: (x + MAGIC) - MAGIC == rint(x) in f32

TRACE = False                # test harness sets True to capture an NTFF profile
LAST_RESULT = None           # BassKernelResults of the most recent run

_PROG_CACHE = {}


def _build_program(s: int):
    import concourse.bacc as bacc
    import concourse.mybir as mybir
    from concourse.tile import TileContext

    f32 = mybir.dt.float32
    f16 = mybir.dt.float16
    op = mybir.AluOpType

    nc = bacc.Bacc("TRN2", target_bir_lowering=False, debug=False,
                   num_devices=NCORES)

    i8 = mybir.dt.int8
    NRR = HSH * B
    cin = nc.dram_tensor("cin", [NT, 128, F], i8, kind="ExternalInput")
    # all six tiles' scales, partition-major -> one DMA, one semaphore
    sc = nc.dram_tensor("sc", [128, NT * CPS], f16, kind="ExternalInput")
    # [ident(16x16) | nk rows ; nv rows (16x128)] -> one DMA, one K+V chain
    aux = nc.dram_tensor("aux", [2 * NRR, 2 * NRR + D], f32,
                         kind="ExternalInput")
    out = nc.dram_tensor("out", [NT, 128, F], f16, kind="ExternalOutput")

    # patch site of the replacement row for each cache: rows [8s, 8s+8) of
    # the cache's 24576 rows; 64-row slabs -> tile, partition, column
    NR = HSH * B                              # 8 rows per seq position
    patch = {}
    for i, nm in enumerate(("k", "v")):
        slab = i * (ROWS // CPS) + (s * NR) // CPS
        t_star, p_star = divmod(slab, 128)
        c0 = (s * NR) % CPS
        patch.setdefault(t_star, []).append((i, p_star, c0))
    patch_tiles = sorted(patch)
    nonpatch = [t for t in range(NT) if t not in patch]
    # patch tiles go LAST: their int8/scale patches (tiny gpsimd DMAs,
    # ready by ~t=20us) then complete long before these tiles are cast,
    # so no store ever waits on a patch-DMA completion semaphore.
    order = nonpatch + patch_tiles

    with TileContext(nc) as tc:
        with tc.tile_pool(name="row", bufs=1) as rowpool, \
             tc.psum_pool(name="ps", bufs=2) as pspool, \
             tc.tile_pool(name="c8", bufs=NT) as c8pool, \
             tc.tile_pool(name="cp", bufs=NT) as cpool, \
             tc.tile_pool(name="sp", bufs=NT) as spool:
            # --- aux rides first on the sync queue (the DVE chain is the
            # critical-path start); scales on the scalar queue
            NP = 2 * NR                          # 16 rows: K then V
            auxt = rowpool.tile([NP, NP + D], f32, tag="aux")
            nc.sync.dma_start(auxt[:], aux[:])
            st_all = spool.tile([128, NT * CPS], f16, tag="st")
            nc.scalar.dma_start(st_all[:], sc[:])

            # --- sync queue: the six bulk loads, back-to-back; the first
            # (the DVE's first cast tile) in halves so its completion
            # semaphore lands before the chain finishes
            c8s = []
            for pos, t in enumerate(order):
                c8 = c8pool.tile([128, F], i8, tag="c8")
                if pos == 0:
                    nc.sync.dma_start(c8[:, 0:F // 2], cin[t, :, 0:F // 2])
                    nc.sync.dma_start(c8[:, F // 2:F], cin[t, :, F // 2:F])
                else:
                    nc.sync.dma_start(c8[:], cin[t])
                c8s.append(c8)

            # --- quantized replacement row (tiny, v1 math) on DVE.  Yields
            # the int8 q row (transposed to (D, NP) via the idle PE so the
            # patch DMA's iteration order matches the d-major tile layout)
            # and the fp16 dequant scale; both are patched into the int8
            # input tile / scale tile BEFORE the bulk cast+mult consumes
            # them, so the replacement row rides the normal dequant path.
            idt = auxt[:, 0:NP]
            rt = auxt[:, NP:NP + D]
            sig = rowpool.tile([NP, 1], f32, tag="sig")
            nc.vector.tensor_reduce(sig[:], rt,
                                    axis=mybir.AxisListType.X,
                                    op=op.max, apply_absolute_value=True)
            rc = rowpool.tile([NP, 1], f32, tag="rc")
            nc.vector.reciprocal(rc[:], sig[:])
            rr = rowpool.tile([NP, 1], f32, tag="rr")
            nc.vector.tensor_scalar(rr[:], rc[:], MAX_INT8, None, op.mult)
            tt = rowpool.tile([NP, D], f32, tag="tt")
            nc.vector.tensor_scalar(tt[:], rt, rr[:], None, op.mult)
            qt = rowpool.tile([NP, D], f32, tag="qt")
            nc.vector.tensor_scalar(qt[:], tt[:], MAGIC, None, op.add)
            s2 = rowpool.tile([NP, 1], f32, tag="s2")
            nc.vector.tensor_scalar(s2[:], sig[:], C_DEQ, None, op.mult)
            s2h = rowpool.tile([NP, 1], f16, tag="s2h")
            nc.vector.tensor_copy(s2h[:], s2[:])
            # q = rint(t), clamped to the int8 range (the per-row abs-max
            # element quantizes to exactly 128; clamping to 127 costs
            # ~0.8% of that single element's value, well inside the gate)
            qm = rowpool.tile([NP, D], f32, tag="qm")
            nc.vector.tensor_scalar(qm[:], qt[:], MAGIC, 127.0,
                                    op.subtract, op.min)
            qc = rowpool.tile([NP, D], f32, tag="qc")
            nc.vector.tensor_scalar(qc[:], qm[:], -127.0, None, op.max)
            ps = pspool.tile([D, NP], f32, tag="ps")
            nc.tensor.transpose(ps[:], qc[:], idt)
            dr8 = rowpool.tile([D, NP], i8, tag="dr8")
            nc.vector.tensor_copy(dr8[:], ps[:])

            # --- int8 + scale patches (tiny gpsimd DMAs).  The int8 q row
            # overwrites the stale cache row inside the loaded input tile,
            # and the new dequant scale overwrites the stale scale entries;
            # the bulk cast+mult then dequantizes the replacement row like
            # any other.  Both complete ~20us before the patch tiles (last
            # in compute order) are consumed — never on the critical path.
            for pos, t in enumerate(order):
                for i, p_star, c0 in patch.get(t, ()):
                    nc.gpsimd.dma_start(
                        st_all[p_star:p_star + 1,
                               t * CPS + c0:t * CPS + c0 + NR],
                        s2h[i * NR:(i + 1) * NR])
            for pos, t in enumerate(order):
                for i, p_star, c0 in patch.get(t, ()):
                    tgt = c8s[pos][p_star:p_star + 1].rearrange(
                        "p (d c) -> p d c", c=CPS)[:, :, c0:c0 + NR]
                    nc.gpsimd.dma_start(tgt, dr8[:, i * NR:(i + 1) * NR])

            # --- per-tile pipeline: cast -> mult -> store.  Act casts four
            # tiles, the DVE casts the other two (interleaved between its
            # multiplies where the schedule has slack).  Stores are issued
            # by the otherwise-idle sync engine; it stalls on each tile's
            # mult semaphore and thereby feeds the sync HWDGE queue in
            # exactly readiness order, directly behind the loads.  All
            # stores are full 128-partition tiles — partition-subrange DMAs
            # are serialized onto a single DMA engine by the HWDGE and
            # head-of-line-block the queue, so they must never be used for
            # bulk data.
            CAST_ENG = {0: "dve", 1: "act", 2: "act", 3: "dve",
                        4: "act", 5: "act"}
            for pos, t in enumerate(order):
                c8 = c8s[pos]
                st = st_all[:, t * CPS:(t + 1) * CPS]
                ct = cpool.tile([128, F], f16, tag="ct")
                eng = CAST_ENG[pos]
                nch = 2 if (eng == "act" or pos == 0) else 1
                dper = D // nch
                for ci in range(nch):
                    fsl = slice(ci * (F // nch), (ci + 1) * (F // nch))
                    if eng == "act":
                        nc.scalar.activation(
                            ct[:, fsl], c8[:, fsl],
                            mybir.ActivationFunctionType.Copy)
                    else:
                        nc.vector.tensor_copy(ct[:, fsl], c8[:, fsl])
                    ct3 = ct[:, fsl].rearrange("p (d c) -> p d c", c=CPS)
                    stb = st.unsqueeze(1).broadcast_to((128, dper, CPS))
                    nc.vector.tensor_tensor(ct3, ct3, stb, op.mult)
                # stores in half-tiles (full 128 partitions, 8KB lines) so
                # the backlog behind the last mult is 1 MiB, not 2 MiB; the
                # final tile goes out in quarters to shrink the tail drain.
                # The last two tiles' stores ride the scalar HWDGE queue
                # (free after the casts): the two queues' completion-
                # semaphore pipelines then drain the end-of-run update
                # backlog in parallel.
                nst = 4 if pos == NT - 1 else 2
                seng = nc.scalar if pos >= NT - 2 else nc.sync
                for h in range(nst):
                    hsl = slice(h * (F // nst), (h + 1) * (F // nst))
                    seng.dma_start(out[t, :, hsl], ct[:, hsl])
    nc.compile()
    return nc


def _prog(s: int):
    if s not in _PROG_CACHE:
        _PROG_CACHE[s] = _build_program(s)
    return _PROG_CACHE[s]


def _to_dmajor(rows16):
    """(24576, 128) fp16 row-major -> (TPC, 128, F) d-major per 64-row slab."""
    a = rows16.reshape(TPC, 128, CPS, D)      # [t, p, c, d]
    return np.ascontiguousarray(a.transpose(0, 1, 3, 2)).reshape(TPC, 128, F)


def _from_dmajor(tiles16):
    """(TPC, 128, F) fp16 d-major -> (24576, 128) f32 row-major."""
    a = tiles16.reshape(TPC, 128, D, CPS).transpose(0, 1, 3, 2)
    return a.astype(np.float32).reshape(ROWS, D)


def kernel(key, value, cached_ar_key, cached_ar_value,
           cached_ar_key_scale, cached_ar_value_scale, ar_cache_index):
    global LAST_RESULT
    from concourse.bass_utils import run_bass_kernel_spmd

    key = np.asarray(key, dtype=np.float32)
    value = np.asarray(value, dtype=np.float32)
    cached_ar_key = np.asarray(cached_ar_key, dtype=np.float32)
    cached_ar_value = np.asarray(cached_ar_value, dtype=np.float32)
    cached_ar_key_scale = np.asarray(cached_ar_key_scale, dtype=np.float32)
    cached_ar_value_scale = np.asarray(cached_ar_value_scale, dtype=np.float32)
    s = int(ar_cache_index)

    nc = _prog(s)

    # int8-valued cache entries: stream the int8 clip through the device,
    # fix up the rare clipped outliers (|q| > 127) exactly on the host
    k8 = np.clip(cached_ar_key, -128, 127).astype(np.int8)
    v8 = np.clip(cached_ar_value, -128, 127).astype(np.int8)
    key_t = np.ascontiguousarray(key[:, 0].transpose(1, 0, 2))      # (H,B,D)
    val_t = np.ascontiguousarray(value[:, 0].transpose(1, 0, 2))

    in_maps = []
    for i in range(NCORES):
        h0 = i * HSH
        hs = slice(h0, h0 + HSH)
        cin = np.empty((NT, 128, F), np.int8)
        cin[:TPC] = _to_dmajor(k8[:, hs].reshape(ROWS, D))
        cin[TPC:] = _to_dmajor(v8[:, hs].reshape(ROWS, D))
        scf = np.empty((NT, 128, CPS), np.float32)
        scf[:TPC] = cached_ar_key_scale[:, hs].reshape(TPC, 128, CPS)
        scf[TPC:] = cached_ar_value_scale[:, hs].reshape(TPC, 128, CPS)
        sc16 = (scf * np.float32(C_DEQ)).astype(np.float16)
        npp = 2 * HSH * B
        aux = np.empty((npp, npp + D), np.float32)
        aux[:, :npp] = np.eye(npp, dtype=np.float32)
        aux[:HSH * B, npp:] = key_t[hs].reshape(HSH * B, D)
        aux[HSH * B:, npp:] = val_t[hs].reshape(HSH * B, D)
        in_maps.append({
            "cin": cin,
            "sc": np.ascontiguousarray(sc16.transpose(1, 0, 2)).reshape(
                128, NT * CPS),
            "aux": aux,
        })

    res = run_bass_kernel_spmd(nc, in_maps, list(range(NCORES)), trace=TRACE)
    LAST_RESULT = res

    k_out = np.empty((S_AR, H, B, D), np.float32)
    v_out = np.empty((S_AR, H, B, D), np.float32)
    for i, r in enumerate(res.results):
        h0 = i * HSH
        o = np.asarray(r["out"])
        k_out[:, h0:h0 + HSH] = _from_dmajor(o[:TPC]).reshape(S_AR, HSH, B, D)
        v_out[:, h0:h0 + HSH] = _from_dmajor(o[TPC:]).reshape(S_AR, HSH, B, D)

    # exact host fixup of int8-clipped outliers (row s comes from the new
    # decode step on device, so its stale cache values are excluded)
    for cache, scale, outa in ((cached_ar_key, cached_ar_key_scale, k_out),
                               (cached_ar_value, cached_ar_value_scale, v_out)):
        mask = np.abs(cache) > 127
        mask[s] = False
        idx = np.nonzero(mask)
        outa[idx] = cache[idx] * (scale[idx[0], idx[1], idx[2], 0]
                                  * np.float32(C_DEQ))
    # same fixup for the new decode-step rows, for the elements the int8
    # path can't reproduce: each row's abs-max element quantizes to
    # exactly +-128 (clamped to +-127 on device), and elements whose
    # scaled value lands within ~1e-3 of a half-integer can rint the
    # other way (the device computes 127.5/sig via reciprocal+multiply,
    # the reference via divide).  Restore those exactly (q * scale/127.5).
    for new, outa in ((key, k_out), (value, v_out)):
        r = new[:, 0]                                     # (B, H, D)
        sig = np.abs(r).max(axis=-1, keepdims=True)
        t = r * (np.float32(MAX_INT8) / sig)
        q = np.rint(t)
        bad = (np.abs(q) > 127) | (np.abs(np.abs(t - q) - 0.5) < 1e-3)
        b_i, h_i, d_i = np.nonzero(bad)
        outa[s, h_i, b_i, d_i] = (q[b_i, h_i, d_i]
                                  * sig[b_i, h_i, 0] * np.float32(C_DEQ))
    return k_out, v_out
